# revision 1
# baseline (speedup 1.0000x reference)
"""Trainium2 Bass kernel for a ConvViT-style dense transformer block.

Reference computation (B=2, N=3136=56x56, C=512, 8 heads, hidden 2048):
    x = x + Attn(LN1(x));  x = x + MLP(LN2(x))
    MLP = fc2(gelu(dwconv3x3(fc1(.)) + dw_b))

Sharding: tokens are sharded 8 ways as (batch, 14-image-row) stripes.
Each core computes attention/MLP for its own 14 rows (plus 1 halo row on
each side for the depthwise conv), recomputing K/V projections for its
full batch locally (no collectives).  Host does the (free) scatter/gather.
"""

import numpy as np

# ---------------- problem constants (hardcoded per spec) ----------------
B = 2
HI = 56          # image rows
WI = 56          # image cols
NB = HI * WI     # tokens per batch = 3136
C = 512
NH = 8
HD = 64
F3 = 3 * C       # 1536
HID = 4 * C      # 2048
EPS = 1e-5
NCORES = 8
RPC = HI // 4    # image rows per core = 14
EXTR = RPC + 2   # rows incl halo = 16
EXT = EXTR * WI  # 896 ext tokens
OWN = RPC * WI   # 784 own tokens
QCH = EXT // 2   # 448 q-chunk

_CACHE = {}


def _btiles():
    # 128-token tiles over the full batch (24 x 128 + 1 x 64)
    return [(i * 128, min(128, NB - i * 128)) for i in range((NB + 127) // 128)]


def _bchunks():
    # 512-token chunks over the full batch (6 x 512 + 1 x 64)
    return [(i * 512, min(512, NB - i * 512)) for i in range((NB + 511) // 512)]


def _build_nc():
    import concourse.bass as bass
    import concourse.bacc as bacc
    import concourse.tile as tile
    from concourse import mybir

    f32 = mybir.dt.float32
    b16 = mybir.dt.bfloat16
    AF = mybir.ActivationFunctionType
    OP = mybir.AluOpType

    nc = bacc.Bacc(trn_type="TRN2")

    # ---- external I/O ----
    xb_d = nc.dram_tensor("xb", [NB, C], f32, kind="ExternalInput")
    xe_d = nc.dram_tensor("xe", [EXT, C], f32, kind="ExternalInput")
    mask_d = nc.dram_tensor("mask", [EXT], b16, kind="ExternalInput")
    qkvT_d = nc.dram_tensor("qkvT", [C, F3], b16, kind="ExternalInput")
    qkvb_d = nc.dram_tensor("qkvb", [1, F3], b16, kind="ExternalInput")
    outT_d = nc.dram_tensor("outT", [C, C], b16, kind="ExternalInput")
    outb_d = nc.dram_tensor("outb", [1, C], b16, kind="ExternalInput")
    fc1T_d = nc.dram_tensor("fc1T", [C, HID], b16, kind="ExternalInput")
    fc1b_d = nc.dram_tensor("fc1b", [1, HID], b16, kind="ExternalInput")
    fc2T_d = nc.dram_tensor("fc2T", [HID, C], b16, kind="ExternalInput")
    fc2b_d = nc.dram_tensor("fc2b", [1, C], b16, kind="ExternalInput")
    dww_d = nc.dram_tensor("dww", [HID, 9], f32, kind="ExternalInput")
    dwb_d = nc.dram_tensor("dwb", [HID], f32, kind="ExternalInput")
    ident_d = nc.dram_tensor("ident", [128, 128], b16, kind="ExternalInput")
    out_d = nc.dram_tensor("out", [OWN, C], f32, kind="ExternalOutput")

    btiles = _btiles()
    bchunks = _bchunks()
    etiles = [(i * 128, 128) for i in range(EXT // 128)]          # 7 x 128
    otiles = [(i * 128, min(128, OWN - i * 128)) for i in range((OWN + 127) // 128)]

    with tile.TileContext(nc) as tc:
        from contextlib import ExitStack

        with ExitStack() as ctx:
            wp = ctx.enter_context(tc.tile_pool(name="wp", bufs=1))
            big = ctx.enter_context(tc.tile_pool(name="big", bufs=1))
            stage = ctx.enter_context(tc.tile_pool(name="stage", bufs=3))
            small = ctx.enter_context(tc.tile_pool(name="small", bufs=4))
            atp = ctx.enter_context(tc.tile_pool(name="atp", bufs=3))
            padp = ctx.enter_context(tc.tile_pool(name="padp", bufs=2))
            pss = ctx.enter_context(tc.tile_pool(name="pss", bufs=2, space="PSUM"))
            pso = ctx.enter_context(tc.tile_pool(name="pso", bufs=2, space="PSUM"))
            _ps_ctr = [0]

            def mk_ps():
                _ps_ctr[0] ^= 1
                t = "oA" if _ps_ctr[0] else "oB"
                return pso.tile([128, 512], f32, tag=t, name=f"ps_{t}")

            # ---------------- constants / weights into SBUF ----------------
            qkvT = wp.tile([128, 4, F3], b16, tag="qkvT")
            nc.sync.dma_start(out=qkvT, in_=qkvT_d[:, :].rearrange("(g p) f -> p g f", p=128))
            qkvb = wp.tile([1, F3], b16, tag="qkvb")
            nc.sync.dma_start(out=qkvb, in_=qkvb_d[:, :])
            outTs = wp.tile([64, 8, C], b16, tag="outTs")
            nc.sync.dma_start(out=outTs, in_=outT_d[:, :].rearrange("(h p) f -> p h f", p=64))
            outb = wp.tile([1, C], b16, tag="outb")
            nc.sync.dma_start(out=outb, in_=outb_d[:, :])
            fc1T = wp.tile([128, 4, HID], b16, tag="fc1T")
            nc.sync.dma_start(out=fc1T, in_=fc1T_d[:, :].rearrange("(g p) f -> p g f", p=128))
            fc1b = wp.tile([1, HID], b16, tag="fc1b")
            nc.sync.dma_start(out=fc1b, in_=fc1b_d[:, :])
            fc2b = wp.tile([1, C], b16, tag="fc2b")
            nc.sync.dma_start(out=fc2b, in_=fc2b_d[:, :])
            dww = wp.tile([128, 16, 9], f32, tag="dww")
            nc.sync.dma_start(out=dww, in_=dww_d[:, :].rearrange("(g p) t -> p g t", p=128))
            dwb = wp.tile([128, 16], f32, tag="dwb")
            nc.sync.dma_start(out=dwb, in_=dwb_d[:].rearrange("(g p) -> p g", p=128))
            maskb = wp.tile([128, EXT], b16, tag="maskb")
            nc.sync.dma_start(
                out=maskb,
                in_=bass.AP(tensor=mask_d[:].tensor, offset=0, ap=[[0, 128], [1, EXT]]),
            )
            ones = wp.tile([1, C], b16, tag="ones")
            nc.vector.memset(ones, 1.0)
            onesq = wp.tile([128, 128], b16, tag="onesq")
            nc.vector.memset(onesq, 1.0)
            epsc = wp.tile([128, 1], f32, tag="epsc")
            nc.vector.memset(epsc, EPS)
            ident = wp.tile([128, 128], b16, tag="ident")
            nc.sync.dma_start(out=ident, in_=ident_d[:, :])

            # ---------------- LN1 over full batch + ext slice ----------------
            def layer_norm_tile(xt, ts, lt):
                st = small.tile([128, 6], f32, tag="st")
                nc.vector.bn_stats(out=st[:ts], in_=xt[:ts])
                mv = small.tile([128, 2], f32, tag="mv")
                nc.vector.bn_aggr(out=mv[:ts], in_=st[:ts])
                nc.scalar.activation(
                    out=mv[:ts, 1:2], in_=mv[:ts, 1:2], func=AF.Sqrt,
                    bias=epsc[:ts], scale=1.0,
                )
                nc.vector.reciprocal(out=mv[:ts, 1:2], in_=mv[:ts, 1:2])
                nc.vector.tensor_scalar(
                    out=lt[:ts], in0=xt[:ts],
                    scalar1=mv[:ts, 0:1], scalar2=mv[:ts, 1:2],
                    op0=OP.subtract, op1=OP.mult,
                )

            def pe_transpose(lt, ts, t0, put):
                # PE-transpose [ts,128] blocks of lt into c-major storage
                for cc in range(4):
                    tp = pss.tile([128, 128], b16, tag="sA" if cc % 2 == 0 else "sB")
                    nc.tensor.transpose(
                        tp[:, :ts], lt[:ts, cc * 128 : (cc + 1) * 128], ident[:ts, :ts]
                    )
                    put(cc, t0, ts, tp)

            def layer_norm_T(src_d, tiles, put):
                for t0, ts in tiles:
                    xt = stage.tile([128, C], f32, tag="xf")
                    nc.gpsimd.dma_start(out=xt[:ts], in_=src_d[t0 : t0 + ts, :])
                    lt = stage.tile([128, C], b16, tag="xl")
                    layer_norm_tile(xt, ts, lt)
                    pe_transpose(lt, ts, t0, put)

            ln1xT = [big.tile([128, NB], b16, tag=f"lx{c}", name=f"ln1xT{c}") for c in range(4)]
            ln1eT = big.tile([128, 4, EXT], b16, tag="le")

            def put_ln1x(cc, t0, ts, tp):
                nc.vector.tensor_copy(out=ln1xT[cc][:, t0 : t0 + ts], in_=tp[:, :ts])

            def put_ln1e(cc, t0, ts, tp):
                nc.vector.tensor_copy(out=ln1eT[:, cc, t0 : t0 + ts], in_=tp[:, :ts])

            layer_norm_T(xb_d, btiles, put_ln1x)
            layer_norm_T(xe_d, etiles, put_ln1e)

            # ---------------- projections: KT, V5, QT ----------------
            KT = [big.tile([128, NB], b16, tag=f"kt{c}", name=f"KT{c}") for c in range(4)]
            for f in range(4):
                for t0, tn in bchunks:
                    ps = mk_ps()
                    for c in range(4):
                        nc.tensor.matmul(
                            ps[:, :tn],
                            qkvT[:, c, C + f * 128 : C + (f + 1) * 128],
                            ln1xT[c][:, t0 : t0 + tn],
                            start=(c == 0), stop=False,
                        )
                    nc.tensor.matmul(
                        ps[:, :tn],
                        qkvb[:, C + f * 128 : C + (f + 1) * 128],
                        ones[:, :tn],
                        start=False, stop=True,
                    )
                    if (t0 // 512) % 2 == 0:
                        nc.vector.tensor_copy(out=KT[f][:, t0 : t0 + tn], in_=ps[:, :tn])
                    else:
                        nc.scalar.activation(
                            out=KT[f][:, t0 : t0 + tn], in_=ps[:, :tn], func=AF.Copy
                        )

            V5 = big.tile([128, len(btiles), 8, 65], b16, tag="v5")
            nc.vector.memset(V5[:, :, :, 64:65], 1.0)
            for i, (t0, ts) in enumerate(btiles):
                ps = mk_ps()
                for c in range(4):
                    nc.tensor.matmul(
                        ps[:ts],
                        ln1xT[c][:, t0 : t0 + ts],
                        qkvT[:, c, 2 * C : 3 * C],
                        start=(c == 0), stop=False,
                    )
                nc.tensor.matmul(
                    ps[:ts], ones[:, :ts], qkvb[:, 2 * C : 3 * C],
                    start=False, stop=True,
                )
                eng_v = i % 2 == 0
                if eng_v:
                    nc.vector.tensor_copy(
                        out=V5[:ts, i, :, 0:64],
                        in_=ps[:ts].rearrange("p (h d) -> p h d", d=64),
                    )
                else:
                    nc.scalar.activation(
                        out=V5[:ts, i, :, 0:64],
                        in_=ps[:ts].rearrange("p (h d) -> p h d", d=64),
                        func=AF.Copy,
                    )

            QT = big.tile([128, 4, EXT], b16, tag="qt")
            for f in range(4):
                for qc in range(2):
                    q0 = qc * QCH
                    ps = mk_ps()
                    for c in range(4):
                        nc.tensor.matmul(
                            ps[:, :QCH],
                            qkvT[:, c, f * 128 : (f + 1) * 128],
                            ln1eT[:, c, q0 : q0 + QCH],
                            start=(c == 0), stop=False,
                        )
                    nc.tensor.matmul(
                        ps[:, :QCH], qkvb[:, f * 128 : (f + 1) * 128],
                        ones[:, :QCH], start=False, stop=True,
                    )
                    nc.vector.tensor_copy(out=QT[:, f, q0 : q0 + QCH], in_=ps[:, :QCH])

            # ---------------- attention ----------------
            oTs = big.tile([64, 8, EXT], b16, tag="oTs")
            srow = big.tile([65, 8, QCH], b16, tag="srow")

            for qc in range(2):
                q0 = qc * QCH
                for pr in range(4):
                    hA, hB = 2 * pr, 2 * pr + 1
                    oA = pso.tile([65, QCH], f32, tag="oA")
                    oB = pso.tile([65, QCH], f32, tag="oB")
                    for kt, (k0, kn) in enumerate(btiles):
                        sA = pss.tile([128, QCH], f32, tag="sA")
                        sB = pss.tile([128, QCH], f32, tag="sB")
                        nc.tensor.matmul(
                            sA[:kn], KT[pr][0:64, k0 : k0 + kn],
                            QT[0:64, pr, q0 : q0 + QCH], start=True, stop=True,
                            tile_position=(0, 0),
                        )
                        nc.tensor.matmul(
                            sB[:kn], KT[pr][64:128, k0 : k0 + kn],
                            QT[64:128, pr, q0 : q0 + QCH], start=True, stop=True,
                            tile_position=(64, 0),
                        )
                        exA = atp.tile([128, QCH], b16, tag="exA")
                        exB = atp.tile([128, QCH], b16, tag="exB")
                        nc.scalar.activation(out=exA[:kn], in_=sA[:kn], func=AF.Exp)
                        nc.scalar.activation(out=exB[:kn], in_=sB[:kn], func=AF.Exp)
                        nc.tensor.matmul(
                            oA, V5[:kn, kt, hA, :], exA[:kn],
                            start=(kt == 0), stop=(kt == len(btiles) - 1),
                        )
                        nc.tensor.matmul(
                            oB, V5[:kn, kt, hB, :], exB[:kn],
                            start=(kt == 0), stop=(kt == len(btiles) - 1),
                        )
                    # stash unnormalized o and the exp-sums (partition 64)
                    nc.vector.tensor_copy(out=oTs[:, hA, q0 : q0 + QCH], in_=oA[0:64])
                    nc.vector.tensor_copy(out=oTs[:, hB, q0 : q0 + QCH], in_=oB[0:64])
                    nc.vector.tensor_copy(out=srow[64:65, hA, :], in_=oA[64:65])
                    nc.vector.tensor_copy(out=srow[64:65, hB, :], in_=oB[64:65])
                # reciprocal of all 8 sums at once: 1/s = exp(-ln(s)), in place
                nc.scalar.activation(
                    out=srow[64:65].rearrange("p a b -> p (a b)"),
                    in_=srow[64:65].rearrange("p a b -> p (a b)"),
                    func=AF.Ln,
                )
                nc.scalar.activation(
                    out=srow[64:65].rearrange("p a b -> p (a b)"),
                    in_=srow[64:65].rearrange("p a b -> p (a b)"),
                    func=AF.Exp, scale=-1.0,
                )
                # broadcast 1/s to 64 partitions and normalize oTs
                for h in range(8):
                    rb = pss.tile([128, QCH], f32, tag="sA")
                    nc.tensor.matmul(
                        rb[0:64], onesq[64:65, 0:64], srow[64:65, h, :],
                        start=True, stop=True,
                    )
                    nc.vector.scalar_tensor_tensor(
                        out=oTs[:, h, q0 : q0 + QCH],
                        in0=oTs[:, h, q0 : q0 + QCH],
                        scalar=1.0, in1=rb[0:64],
                        op0=OP.bypass, op1=OP.mult,
                    )

            # ---------------- out-proj + residual + LN2 ----------------
            a_sb = big.tile([128, 7, C], f32, tag="a_sb")
            ln2aT = big.tile([128, 4, EXT], b16, tag="le")  # reuse ln1eT slot

            def put_ln2a(cc, t0, ts, tp):
                nc.vector.tensor_copy(out=ln2aT[:, cc, t0 : t0 + ts], in_=tp[:, :ts])

            for i, (t0, ts) in enumerate(etiles):
                ps = mk_ps()
                for h in range(8):
                    nc.tensor.matmul(
                        ps, oTs[:, h, t0 : t0 + ts], outTs[:, h, :],
                        start=(h == 0), stop=False,
                    )
                nc.tensor.matmul(ps, ones[:, :ts], outb, start=False, stop=True)
                xt = stage.tile([128, C], f32, tag="xf")
                nc.gpsimd.dma_start(out=xt[:ts], in_=xe_d[t0 : t0 + ts, :])
                nc.vector.tensor_add(out=a_sb[:ts, i, :], in0=xt[:ts], in1=ps[:ts])
                lt = stage.tile([128, C], b16, tag="xl")
                layer_norm_tile(a_sb[:, i, :], ts, lt)
                pe_transpose(lt, ts, t0, put_ln2a)

            # ---------------- MLP: fc1 -> dwconv+mask -> gelu -> fc2 ----------------
            # fc2 weights arrive late, into the slots KT vacated after attention
            fc2Ta = big.tile([128, 8, C], b16, tag="kt0")
            nc.gpsimd.dma_start(
                out=fc2Ta, in_=fc2T_d[0:1024, :].rearrange("(g p) f -> p g f", p=128)
            )
            fc2Tb = big.tile([128, 8, C], b16, tag="kt1")
            nc.gpsimd.dma_start(
                out=fc2Tb, in_=fc2T_d[1024:2048, :].rearrange("(g p) f -> p g f", p=128)
            )
            ghT = [big.tile([128, 4, OWN], b16, tag=f"lx{k}", name=f"ghT{k}") for k in range(4)]
            SPAN = RPC * (WI + 2)          # 812 flat conv span (2 junk cols/row)
            PADW = EXTR * (WI + 2) + 2     # 930: +2 so the last tap's junk reads stay in-bounds
            for g in range(16):
                pad = padp.tile([128, PADW], b16, tag="pad")
                padv = pad[:, : PADW - 2].rearrange("p (r x) -> p r x", x=WI + 2)
                nc.vector.memset(pad[:, PADW - 2 :], 0.0)
                nc.vector.memset(padv[:, :, 0:1], 0.0)
                nc.vector.memset(padv[:, :, WI + 1 : WI + 2], 0.0)
                for qc in range(2):
                    q0 = qc * QCH
                    ps = mk_ps()
                    for c in range(4):
                        nc.tensor.matmul(
                            ps[:, :QCH],
                            fc1T[:, c, g * 128 : (g + 1) * 128],
                            ln2aT[:, c, q0 : q0 + QCH],
                            start=(c == 0), stop=False,
                        )
                    nc.tensor.matmul(
                        ps[:, :QCH], fc1b[:, g * 128 : (g + 1) * 128],
                        ones[:, :QCH], start=False, stop=True,
                    )
                    # masked scatter into padded conv buffer (rows qc*8..qc*8+8)
                    nc.vector.scalar_tensor_tensor(
                        out=padv[:, qc * 8 : (qc + 1) * 8, 1 : WI + 1],
                        in0=ps[:, :QCH].rearrange("p (r x) -> p r x", x=WI),
                        scalar=1.0,
                        in1=maskb[:, q0 : q0 + QCH].rearrange("p (r x) -> p r x", x=WI),
                        op0=OP.bypass, op1=OP.mult,
                    )
                # 3x3 depthwise conv, flat contiguous spans.
                # Even-offset taps on DVE (2x mode), dx==1 taps on GpSimd.
                accD = padp.tile([128, SPAN], b16, tag="accD")
                accP = padp.tile([128, SPAN], b16, tag="accP", bufs=1)
                tmpP = padp.tile([128, SPAN], b16, tag="tmpP", bufs=1)
                firstD = True
                firstP = True
                for dy in range(3):
                    for dx in range(3):
                        tap = 3 * dy + dx
                        off = dy * (WI + 2) + dx
                        shifted = pad[:, off : off + SPAN]
                        wsl = dww[:, g, tap : tap + 1]
                        if dx == 1:
                            wb = bass.AP(
                                tensor=wsl.tensor, offset=wsl.offset,
                                ap=[list(wsl.ap[0]), [0, SPAN]],
                            )
                            if firstP:
                                nc.gpsimd.tensor_tensor(
                                    out=accP, in0=shifted, in1=wb, op=OP.mult
                                )
                                firstP = False
                            else:
                                nc.gpsimd.tensor_tensor(
                                    out=tmpP, in0=shifted, in1=wb, op=OP.mult
                                )
                                nc.gpsimd.tensor_tensor(
                                    out=accP, in0=accP, in1=tmpP, op=OP.add
                                )
                        elif firstD:
                            nc.vector.tensor_scalar_mul(
                                out=accD, in0=shifted, scalar1=wsl
                            )
                            firstD = False
                        else:
                            nc.vector.scalar_tensor_tensor(
                                out=accD, in0=shifted, scalar=wsl,
                                in1=accD, op0=OP.mult, op1=OP.add,
                            )
                nc.vector.tensor_tensor(out=accD, in0=accD, in1=accP, op=OP.add)
                nc.scalar.activation(
                    out=ghT[g // 4][:, g % 4, :],
                    in_=accD.rearrange("p (r x) -> p r x", x=WI + 2)[:, :, 0:WI],
                    func=AF.Gelu, bias=dwb[:, g : g + 1], scale=1.0,
                )

            # ---------------- fc2 + final residual ----------------
            for i, (t0, ts) in enumerate(otiles):
                ps = mk_ps()
                for k in range(16):
                    f2 = fc2Ta[:, k, :] if k < 8 else fc2Tb[:, k - 8, :]
                    nc.tensor.matmul(
                        ps[:ts],
                        ghT[k // 4][:, k % 4, t0 : t0 + ts],
                        f2,
                        start=(k == 0), stop=False,
                    )
                nc.tensor.matmul(ps[:ts], ones[:, :ts], fc2b, start=False, stop=True)
                at = stage.tile([128, C], f32, tag="xf")
                n1 = min(ts, 128 - WI)  # rows from a tile i (partitions WI..)
                nc.gpsimd.dma_start(out=at[:n1], in_=a_sb[WI : WI + n1, i, :])
                if ts > n1:
                    nc.gpsimd.dma_start(
                        out=at[n1:ts], in_=a_sb[0 : ts - n1, i + 1, :]
                    )
                ot = stage.tile([128, C], f32, tag="xa")
                nc.vector.tensor_add(out=ot[:ts], in0=at[:ts], in1=ps[:ts])
                nc.gpsimd.dma_start(out=out_d[t0 : t0 + ts, :], in_=ot[:ts])

    return nc


def _prep_host(inputs):
    import ml_dtypes

    bf16 = ml_dtypes.bfloat16
    f32 = np.float32

    g = {k: np.asarray(v) for k, v in inputs.items()}
    x = g["x"].astype(f32)
    ln1_w, ln1_b = g["ln1_w"].astype(f32), g["ln1_b"].astype(f32)
    ln2_w, ln2_b = g["ln2_w"].astype(f32), g["ln2_b"].astype(f32)
    qkv_w, qkv_b = g["qkv_w"].astype(f32), g["qkv_b"].astype(f32)
    out_w, out_b = g["out_w"].astype(f32), g["out_b"].astype(f32)
    fc1_w, fc1_b = g["fc1_w"].astype(f32), g["fc1_b"].astype(f32)
    fc2_w, fc2_b = g["fc2_w"].astype(f32), g["fc2_b"].astype(f32)
    dw_w, dw_b = g["dw_w"].astype(f32), g["dw_b"].astype(f32)
    temp = float(np.asarray(g["temperature"]))

    # fold LN affine into the following matmul; fold 1/temperature into W_q
    qkv_w2 = qkv_w * ln1_w[None, :]
    qkv_b2 = qkv_b + qkv_w @ ln1_b
    qkv_w2[:C] /= temp
    qkv_b2[:C] /= temp
    fc1_w2 = fc1_w * ln2_w[None, :]
    fc1_b2 = fc1_b + fc1_w @ ln2_b

    shared = {
        "qkvT": np.ascontiguousarray(qkv_w2.T).astype(bf16),
        "qkvb": qkv_b2[None, :].astype(bf16),
        "outT": np.ascontiguousarray(out_w.T).astype(bf16),
        "outb": out_b[None, :].astype(bf16),
        "fc1T": np.ascontiguousarray(fc1_w2.T).astype(bf16),
        "fc1b": fc1_b2[None, :].astype(bf16),
        "fc2T": np.ascontiguousarray(fc2_w.T).astype(bf16),
        "fc2b": fc2_b[None, :].astype(bf16),
        "dww": np.ascontiguousarray(dw_w.reshape(HID, 9)).astype(f32),
        "dwb": dw_b.astype(f32),
        "ident": np.eye(128, dtype=f32).astype(bf16),
    }

    ximg = x.reshape(B, HI, WI, C)
    in_maps = []
    for c in range(NCORES):
        b, qi = c // 4, c % 4
        r0 = RPC * qi
        xe = np.zeros((EXTR, WI, C), f32)
        mask = np.zeros((EXTR, WI), f32)
        for e in range(EXTR):
            r = r0 - 1 + e
            if 0 <= r < HI:
                xe[e] = ximg[b, r]
                mask[e] = 1.0
        m = dict(shared)
        m["xb"] = np.ascontiguousarray(x[b])
        m["xe"] = np.ascontiguousarray(xe.reshape(EXT, C))
        m["mask"] = mask.reshape(EXT).astype(bf16)
        in_maps.append(m)
    return in_maps


def _run(inputs, trace=False):
    from concourse.bass_utils import run_bass_kernel_spmd

    if "nc" not in _CACHE:
        nc = _build_nc()
        nc.finalize()
        _CACHE["nc"] = nc
    nc = _CACHE["nc"]
    in_maps = _prep_host(inputs)
    res = run_bass_kernel_spmd(nc, in_maps, core_ids=list(range(NCORES)), trace=trace)

    x = np.asarray(inputs["x"])
    out = np.zeros((B, NB, C), np.float32)
    for c in range(NCORES):
        b, qi = c // 4, c % 4
        r0 = RPC * qi
        out[b, r0 * WI : (r0 + RPC) * WI, :] = res.results[c]["out"]
    return out.astype(x.dtype, copy=False), res


def kernel(**inputs) -> np.ndarray:
    out, _ = _run(inputs, trace=False)
    return out



# revision 14
# speedup vs baseline: 1.0409x; 1.0409x over previous
"""Trainium2 Bass kernel for a ConvViT-style dense transformer block.

Reference computation (B=2, N=3136=56x56, C=512, 8 heads, hidden 2048):
    x = x + Attn(LN1(x));  x = x + MLP(LN2(x))
    MLP = fc2(gelu(dwconv3x3(fc1(.)) + dw_b))

Sharding: tokens are sharded 8 ways as (batch, 14-image-row) stripes.
Each core computes attention/MLP for its own 14 rows (plus 1 halo row on
each side for the depthwise conv), recomputing K/V projections for its
full batch locally (no collectives).  Host does the (free) scatter/gather.

Key engine assignment:
  - scores/PV/projections/fc1/fc2 and the 3x3 depthwise conv (as 9
    accumulating diagonal matmuls) run on the PE.
  - softmax exp runs on ACT as fused multi-bank [128, ~1536] activations.
  - LN applies run on ACT (Identity with per-partition scale/bias).
  - softmax 1/sum is folded into the PSUM->SBUF copy (DVE reciprocal +
    DMA partition-broadcast).
"""

import numpy as np

# ---------------- problem constants (hardcoded per spec) ----------------
B = 2
HI = 56          # image rows
WI = 56          # image cols
NB = HI * WI     # tokens per batch = 3136
C = 512
NH = 8
HD = 64
F3 = 3 * C       # 1536
HID = 4 * C      # 2048
EPS = 1e-5
NCORES = 8
RPC = HI // 4    # image rows per core = 14
EXTR = RPC + 2   # rows incl halo = 16
EXT = EXTR * WI  # 896 ext tokens
OWN = RPC * WI   # 784 own tokens
QCHS = [(0, 512), (512, 384)]       # attention q-chunks (bank-aligned)
FCHS = [(0, 504), (504, 392)]       # fc1 chunks: rows 0-8 / 9-15

_CACHE = {}


def _btiles():
    # 128-token tiles over the full batch (24 x 128 + 1 x 64)
    return [(i * 128, min(128, NB - i * 128)) for i in range((NB + 127) // 128)]


def _bchunks():
    # 512-token chunks over the full batch (6 x 512 + 1 x 64)
    return [(i * 512, min(512, NB - i * 512)) for i in range((NB + 511) // 512)]


def _build_nc():
    import concourse.bass as bass
    import concourse.bacc as bacc
    import concourse.tile as tile
    from concourse import mybir

    f32 = mybir.dt.float32
    b16 = mybir.dt.bfloat16
    AF = mybir.ActivationFunctionType
    OP = mybir.AluOpType

    nc = bacc.Bacc(trn_type="TRN2")

    # ---- external I/O ----
    xb_d = nc.dram_tensor("xb", [NB, C], f32, kind="ExternalInput")
    xe_d = nc.dram_tensor("xe", [EXT, C], f32, kind="ExternalInput")
    mask_d = nc.dram_tensor("mask", [EXT], b16, kind="ExternalInput")
    qkvT_d = nc.dram_tensor("qkvT", [C, F3], b16, kind="ExternalInput")
    qkvb_d = nc.dram_tensor("qkvb", [1, F3], b16, kind="ExternalInput")
    outT_d = nc.dram_tensor("outT", [C, C], b16, kind="ExternalInput")
    outb_d = nc.dram_tensor("outb", [1, C], b16, kind="ExternalInput")
    fc1T_d = nc.dram_tensor("fc1T", [C, HID], b16, kind="ExternalInput")
    fc1b_d = nc.dram_tensor("fc1b", [1, HID], b16, kind="ExternalInput")
    fc2T_d = nc.dram_tensor("fc2T", [HID, C], b16, kind="ExternalInput")
    fc2b_d = nc.dram_tensor("fc2b", [1, C], b16, kind="ExternalInput")
    dww_d = nc.dram_tensor("dww", [HID, 9], f32, kind="ExternalInput")
    dwb_d = nc.dram_tensor("dwb", [HID], f32, kind="ExternalInput")
    ident_d = nc.dram_tensor("ident", [128, 128], b16, kind="ExternalInput")
    out_d = nc.dram_tensor("out", [OWN, C], f32, kind="ExternalOutput")

    btiles = _btiles()
    bchunks = _bchunks()
    etiles = [(i * 128, 128) for i in range(EXT // 128)]          # 7 x 128
    otiles = [(i * 128, min(128, OWN - i * 128)) for i in range((OWN + 127) // 128)]

    with tile.TileContext(nc) as tc:
        from contextlib import ExitStack

        with ExitStack() as ctx:
            wp = ctx.enter_context(tc.tile_pool(name="wp", bufs=1))
            big = ctx.enter_context(tc.tile_pool(name="big", bufs=1))
            stage = ctx.enter_context(tc.tile_pool(name="stage", bufs=3))
            small = ctx.enter_context(tc.tile_pool(name="small", bufs=4))
            exr = ctx.enter_context(tc.tile_pool(name="exr", bufs=2))
            padp = ctx.enter_context(tc.tile_pool(name="padp", bufs=2))
            dgp = ctx.enter_context(tc.tile_pool(name="dgp", bufs=2))
            # PSUM: score-group ring 2x3 banks + o accumulators 2x1 bank
            psg = ctx.enter_context(tc.tile_pool(name="psg", bufs=2, space="PSUM"))
            pso = ctx.enter_context(tc.tile_pool(name="pso", bufs=1, space="PSUM"))
            _ps_ctr = [0]

            def mk_ps():
                # general-purpose [128,512] psum (projections, out-proj, fc1,
                # fc2, conv) — rotates over the score-group ring's banks,
                # which are free outside the attention inner loop.
                g = psg.tile([128, 1536], f32, tag="sg", name="ps_g")
                return g[:, 0:512]

            # ---------------- constants / weights into SBUF ----------------
            qkvT = wp.tile([128, 4, F3], b16, tag="qkvT")
            nc.sync.dma_start(out=qkvT, in_=qkvT_d[:, :].rearrange("(g p) f -> p g f", p=128))
            qkvb = wp.tile([1, F3], b16, tag="qkvb")
            nc.sync.dma_start(out=qkvb, in_=qkvb_d[:, :])
            outTs = wp.tile([64, 8, C], b16, tag="outTs")
            nc.sync.dma_start(out=outTs, in_=outT_d[:, :].rearrange("(h p) f -> p h f", p=64))
            outb = wp.tile([1, C], b16, tag="outb")
            nc.sync.dma_start(out=outb, in_=outb_d[:, :])
            fc1T = wp.tile([128, 4, HID], b16, tag="fc1T")
            nc.sync.dma_start(out=fc1T, in_=fc1T_d[:, :].rearrange("(g p) f -> p g f", p=128))
            fc1b = wp.tile([1, HID], b16, tag="fc1b")
            nc.sync.dma_start(out=fc1b, in_=fc1b_d[:, :])
            fc2b = wp.tile([1, C], b16, tag="fc2b")
            nc.sync.dma_start(out=fc2b, in_=fc2b_d[:, :])
            dww = wp.tile([128, 16, 9], f32, tag="dww")
            nc.sync.dma_start(out=dww, in_=dww_d[:, :].rearrange("(g p) t -> p g t", p=128))
            dwb = wp.tile([128, 16], f32, tag="dwb")
            nc.sync.dma_start(out=dwb, in_=dwb_d[:].rearrange("(g p) -> p g", p=128))
            maskb = wp.tile([128, EXT], b16, tag="maskb")
            nc.sync.dma_start(
                out=maskb,
                in_=bass.AP(tensor=mask_d[:].tensor, offset=0, ap=[[0, 128], [1, EXT]]),
            )
            ones = wp.tile([1, C], b16, tag="ones")
            nc.vector.memset(ones, 1.0)
            epsc = wp.tile([128, 1], f32, tag="epsc")
            nc.vector.memset(epsc, EPS)
            ident = wp.tile([128, 128], b16, tag="ident")
            nc.sync.dma_start(out=ident, in_=ident_d[:, :])

            # ---------------- LN helpers ----------------
            def layer_norm_tile(xt, ts, lt):
                # stats on DVE; rstd via ACT sqrt + DVE reciprocal;
                # apply on ACT (Identity with per-partition scale/bias)
                st = small.tile([128, 6], f32, tag="st")
                nc.vector.bn_stats(out=st[:ts], in_=xt[:ts])
                mv = small.tile([128, 4], f32, tag="mv")
                nc.vector.bn_aggr(out=mv[:ts, 0:2], in_=st[:ts])
                nc.scalar.activation(
                    out=mv[:ts, 1:2], in_=mv[:ts, 1:2], func=AF.Sqrt,
                    bias=epsc[:ts], scale=1.0,
                )
                nc.vector.reciprocal(out=mv[:ts, 1:2], in_=mv[:ts, 1:2])
                # mv[:,2] = -mu * rstd
                nc.vector.scalar_tensor_tensor(
                    out=mv[:ts, 2:3], in0=mv[:ts, 0:1], scalar=-1.0,
                    in1=mv[:ts, 1:2], op0=OP.mult, op1=OP.mult,
                )
                nc.scalar.activation(
                    out=lt[:ts], in_=xt[:ts], func=AF.Identity,
                    bias=mv[:ts, 2:3], scale=mv[:ts, 1:2],
                )

            def pe_transpose(lt, ts, t0, put):
                # PE-transpose [ts,128] blocks of lt into c-major storage
                for cc in range(4):
                    tp = psg.tile([128, 128], b16, tag="sg", name="tp")
                    nc.tensor.transpose(
                        tp[:, :ts], lt[:ts, cc * 128 : (cc + 1) * 128], ident[:ts, :ts]
                    )
                    put(cc, t0, ts, tp)

            # ---------------- LN1 + K/V projections, interleaved ----------------
            ln1xT = [big.tile([128, NB], b16, tag=f"lx{c}", name=f"ln1xT{c}") for c in range(4)]
            ln1eT = big.tile([128, 4, EXT], b16, tag="le")
            KT = [big.tile([128, NB], b16, tag=f"kt{c}", name=f"KT{c}") for c in range(4)]
            V5 = big.tile([128, len(btiles), 8, 65], b16, tag="v5")
            nc.vector.memset(V5[:, :, :, 64:65], 1.0)

            def put_ln1x(cc, t0, ts, tp):
                nc.vector.tensor_copy(out=ln1xT[cc][:, t0 : t0 + ts], in_=tp[:, :ts])

            def put_ln1e(cc, t0, ts, tp):
                nc.vector.tensor_copy(out=ln1eT[:, cc, t0 : t0 + ts], in_=tp[:, :ts])

            def ln1_tile(src_d, t0, ts, put):
                xt = stage.tile([128, C], f32, tag="xf")
                nc.gpsimd.dma_start(out=xt[:ts], in_=src_d[t0 : t0 + ts, :])
                lt = stage.tile([128, C], b16, tag="xl")
                layer_norm_tile(xt, ts, lt)
                pe_transpose(lt, ts, t0, put)

            def k_proj_chunk(t0, tn):
                for f in range(4):
                    ps = mk_ps()
                    for c in range(4):
                        nc.tensor.matmul(
                            ps[:, :tn],
                            qkvT[:, c, C + f * 128 : C + (f + 1) * 128],
                            ln1xT[c][:, t0 : t0 + tn],
                            start=(c == 0), stop=False,
                        )
                    nc.tensor.matmul(
                        ps[:, :tn],
                        qkvb[:, C + f * 128 : C + (f + 1) * 128],
                        ones[:, :tn],
                        start=False, stop=True,
                    )
                    if f % 2 == 0:
                        nc.vector.tensor_copy(out=KT[f][:, t0 : t0 + tn], in_=ps[:, :tn])
                    else:
                        nc.scalar.activation(
                            out=KT[f][:, t0 : t0 + tn], in_=ps[:, :tn], func=AF.Copy
                        )

            def v_proj_tile(i, t0, ts):
                ps = mk_ps()
                for c in range(4):
                    nc.tensor.matmul(
                        ps[:ts],
                        ln1xT[c][:, t0 : t0 + ts],
                        qkvT[:, c, 2 * C : 3 * C],
                        start=(c == 0), stop=False,
                    )
                nc.tensor.matmul(
                    ps[:ts], ones[:, :ts], qkvb[:, 2 * C : 3 * C],
                    start=False, stop=True,
                )
                if i % 2 == 0:
                    nc.vector.tensor_copy(
                        out=V5[:ts, i, :, 0:64],
                        in_=ps[:ts].rearrange("p (h d) -> p h d", d=64),
                    )
                else:
                    nc.scalar.activation(
                        out=V5[:ts, i, :, 0:64],
                        in_=ps[:ts].rearrange("p (h d) -> p h d", d=64),
                        func=AF.Copy,
                    )

            # interleave: LN1 batch tiles for chunk c, then K/V proj of chunk c
            for ci, (c0, cn) in enumerate(bchunks):
                for t0, ts in btiles:
                    if c0 <= t0 < c0 + cn:
                        ln1_tile(xb_d, t0, ts, put_ln1x)
                k_proj_chunk(c0, cn)
                for i, (t0, ts) in enumerate(btiles):
                    if c0 <= t0 < c0 + cn:
                        v_proj_tile(i, t0, ts)

            # LN1 on ext tokens + Q projection
            for t0, ts in etiles:
                ln1_tile(xe_d, t0, ts, put_ln1e)

            QT = big.tile([128, 4, EXT], b16, tag="qt")
            for f in range(4):
                for q0, qn in QCHS:
                    ps = mk_ps()
                    for c in range(4):
                        nc.tensor.matmul(
                            ps[:, :qn],
                            qkvT[:, c, f * 128 : (f + 1) * 128],
                            ln1eT[:, c, q0 : q0 + qn],
                            start=(c == 0), stop=False,
                        )
                    nc.tensor.matmul(
                        ps[:, :qn], qkvb[:, f * 128 : (f + 1) * 128],
                        ones[:, :qn], start=False, stop=True,
                    )
                    nc.vector.tensor_copy(out=QT[:, f, q0 : q0 + qn], in_=ps[:, :qn])

            # ---------------- attention ----------------
            # Per (qc, pr): stream of 50 score-matmul outputs (kt-major,
            # head A then B) packed into 3-bank psum group tiles; one fused
            # exp per group; PV matmuls consume the bf16 exp output.
            oTs = big.tile([64, 8, EXT], b16, tag="oTs")
            a_sb = big.tile([128, 7, C], b16, tag="a_sb")
            # reuses ln1eT's slot — dead after the Q projection
            ln2aT = big.tile([128, 4, EXT], b16, tag="le")

            def put_ln2a(cc, t0, ts, tp):
                nc.vector.tensor_copy(out=ln2aT[:, cc, t0 : t0 + ts], in_=tp[:, :ts])

            def outproj_etile(i, t0, ts):
                # out-proj + residual only; LN2 happens post-attention so no
                # ACT table switch lands inside the exp window.
                ps = mk_ps()
                for h in range(8):
                    nc.tensor.matmul(
                        ps, oTs[:, h, t0 : t0 + ts], outTs[:, h, :],
                        start=(h == 0), stop=False,
                    )
                nc.tensor.matmul(ps, ones[:, :ts], outb, start=False, stop=True)
                xt = stage.tile([128, C], f32, tag="xf")
                nc.gpsimd.dma_start(out=xt[:ts], in_=xe_d[t0 : t0 + ts, :])
                nc.vector.tensor_add(out=a_sb[:ts, i, :], in0=xt[:ts], in1=ps[:ts])

            def ln2_etile(i, t0, ts):
                lt = stage.tile([128, C], b16, tag="xl")
                layer_norm_tile(a_sb[:, i, :], ts, lt)
                pe_transpose(lt, ts, t0, put_ln2a)

            nkt = len(btiles)
            for qc, (q0, qn) in enumerate(QCHS):
                for pr in range(4):
                    hA, hB = 2 * pr, 2 * pr + 1
                    oA = pso.tile([128, 512], f32, tag="oA")
                    oB = pso.tile([128, 512], f32, tag="oB")
                    # out j = 2*kt + (0 if head A else 1); groups of 3
                    njs = 2 * nkt
                    ngrp = (njs + 2) // 3
                    grp_tiles = {}
                    ex_tiles = {}
                    for g in range(ngrp):
                        js = list(range(3 * g, min(3 * g + 3, njs)))
                        # score matmuls for this group's outputs
                        for j in js:
                            kt, hb = j // 2, j % 2
                            k0, kn = btiles[kt]
                            if j % 3 == 0:
                                grp_tiles[g] = psg.tile(
                                    [128, 1536], f32, tag="sg", name="sg",
                                )
                            gt = grp_tiles[j // 3]
                            col = (j % 3) * 512
                            nc.tensor.matmul(
                                gt[0:kn, col : col + qn],
                                KT[pr][64 * hb : 64 * hb + 64, k0 : k0 + kn],
                                QT[64 * hb : 64 * hb + 64, pr, q0 : q0 + qn],
                                start=True, stop=True,
                                tile_position=(64 * hb, 0),
                            )
                        # fused exp over the whole group (3D AP skips pad)
                        gt = grp_tiles[g]
                        nj = len(js)
                        ex = exr.tile([128, 1536], b16, tag="ex", name="ex")
                        ex_tiles[g] = ex
                        gv = gt.rearrange("p (s c) -> p s c", c=512)[:, 0:nj, 0:qn]
                        xv = ex.rearrange("p (s c) -> p s c", c=512)[:, 0:nj, 0:qn]
                        nc.scalar.activation(out=xv, in_=gv, func=AF.Exp)
                        # PV matmuls for this group's outputs
                        for j in js:
                            kt, hb = j // 2, j % 2
                            k0, kn = btiles[kt]
                            col = (j % 3) * 512
                            o = oB if hb else oA
                            nc.tensor.matmul(
                                o[0:65, 0:qn],
                                V5[:kn, kt, 2 * pr + hb, :],
                                ex_tiles[j // 3][0:kn, col : col + qn],
                                start=(kt == 0), stop=(kt == nkt - 1),
                            )
                    # fold softmax 1/sum into the o copy-out:
                    # DVE reciprocal of row 64 (the exp-sums), PE-broadcast
                    # to 64 partitions, multiply during PSUM->SBUF copy.
                    for hb, o in ((0, oA), (1, oB)):
                        rstb = small.tile([1, 512], b16, tag="rstb")
                        with nc.allow_low_precision(reason="1/softmax-sum in bf16"):
                            nc.vector.reciprocal(
                                out=rstb[:, 0:qn], in_=o[64:65, 0:qn]
                            )
                        rb = mk_ps()
                        nc.tensor.matmul(
                            rb[0:64, 0:qn], ones[:, 0:64], rstb[:, 0:qn],
                            start=True, stop=True,
                        )
                        dst = oTs[:, 2 * pr + hb, q0 : q0 + qn]
                        nc.vector.tensor_copy(out=dst, in_=o[0:64, 0:qn])
                        nc.vector.scalar_tensor_tensor(
                            out=dst, in0=dst, scalar=1.0, in1=rb[0:64, 0:qn],
                            op0=OP.bypass, op1=OP.mult,
                        )
                    # interleave out-proj of qc0's etiles into the
                    # attention-qc1 window (they fill PE gaps at pr edges)
                    if qc == 1:
                        outproj_etile(pr, *etiles[pr])

            for i in range(4, 7):
                outproj_etile(i, *etiles[i])
            for i in range(7):
                ln2_etile(i, *etiles[i])

            # ---------------- MLP: fc1 -> scatter -> PE dwconv -> gelu ----------------
            # fc2 weights arrive late, into slots KT vacated after attention
            fc2Ta = big.tile([128, 8, C], b16, tag="kt0")
            nc.gpsimd.dma_start(
                out=fc2Ta, in_=fc2T_d[0:1024, :].rearrange("(g p) f -> p g f", p=128)
            )
            fc2Tb = big.tile([128, 8, C], b16, tag="kt1")
            nc.gpsimd.dma_start(
                out=fc2Tb, in_=fc2T_d[1024:2048, :].rearrange("(g p) f -> p g f", p=128)
            )
            ghT = [big.tile([128, 4, OWN], b16, tag=f"lx{k}", name=f"ghT{k}") for k in range(4)]
            SPAN = RPC * (WI + 2)          # 812 flat conv span (2 junk cols/row)
            HSPAN = SPAN // 2              # 406 = 7 rows
            PADW = EXTR * (WI + 2) + 2     # 930: +2 guard for last-tap reads
            for g in range(16):
                pad = padp.tile([128, PADW], b16, tag="pad")
                padv = pad[:, : PADW - 2].rearrange("p (r x) -> p r x", x=WI + 2)
                nc.vector.memset(pad[:, PADW - 2 :], 0.0)
                nc.vector.memset(padv[:, :, 0:1], 0.0)
                nc.vector.memset(padv[:, :, WI + 1 : WI + 2], 0.0)
                for fi, (f0, fn) in enumerate(FCHS):
                    ps = mk_ps()
                    for c in range(4):
                        nc.tensor.matmul(
                            ps[:, :fn],
                            fc1T[:, c, g * 128 : (g + 1) * 128],
                            ln2aT[:, c, f0 : f0 + fn],
                            start=(c == 0), stop=False,
                        )
                    nc.tensor.matmul(
                        ps[:, :fn], fc1b[:, g * 128 : (g + 1) * 128],
                        ones[:, :fn], start=False, stop=True,
                    )
                    r0 = f0 // WI
                    nr = fn // WI
                    nc.vector.scalar_tensor_tensor(
                        out=padv[:, r0 : r0 + nr, 1 : WI + 1],
                        in0=ps[:, :fn].rearrange("p (r x) -> p r x", x=WI),
                        scalar=1.0,
                        in1=maskb[:, f0 : f0 + fn].rearrange("p (r x) -> p r x", x=WI),
                        op0=OP.bypass, op1=OP.mult,
                    )
                # diagonal weight matrices for this group's 9 taps
                dg = dgp.tile([128, 9, 128], b16, tag="dg")
                for tap in range(9):
                    nc.vector.tensor_scalar_mul(
                        out=dg[:, tap, :], in0=ident, scalar1=dww[:, g, tap : tap + 1]
                    )
                # 3x3 depthwise conv: 9 accumulating diag matmuls per chunk
                for ch in range(2):
                    cps = psg.tile([128, 1536], f32, tag="sg", name="cps")
                    for dy in range(3):
                        for dx in range(3):
                            tap = 3 * dy + dx
                            off = dy * (WI + 2) + dx + ch * HSPAN
                            nc.tensor.matmul(
                                cps[:, 0:HSPAN],
                                dg[:, tap, :],
                                pad[:, off : off + HSPAN],
                                start=(tap == 0), stop=(tap == 8),
                            )
                    # gelu(conv + dwb) straight out of PSUM, skipping the
                    # 2 junk cols per row
                    cv = cps[:, 0:HSPAN].rearrange("p (r x) -> p r x", x=WI + 2)
                    nc.scalar.activation(
                        out=ghT[g // 4][:, g % 4, ch * 392 : (ch + 1) * 392],
                        in_=cv[:, :, 0:WI],
                        func=AF.Gelu, bias=dwb[:, g : g + 1], scale=1.0,
                    )

            # ---------------- fc2 + final residual ----------------
            for i, (t0, ts) in enumerate(otiles):
                ps = mk_ps()
                for k in range(16):
                    f2 = fc2Ta[:, k, :] if k < 8 else fc2Tb[:, k - 8, :]
                    nc.tensor.matmul(
                        ps[:ts],
                        ghT[k // 4][:, k % 4, t0 : t0 + ts],
                        f2,
                        start=(k == 0), stop=False,
                    )
                nc.tensor.matmul(ps[:ts], ones[:, :ts], fc2b, start=False, stop=True)
                at = stage.tile([128, C], f32, tag="xf")
                n1 = min(ts, 128 - WI)  # rows from a tile i (partitions WI..)
                nc.gpsimd.dma_start(out=at[:n1], in_=a_sb[WI : WI + n1, i, :])
                if ts > n1:
                    nc.gpsimd.dma_start(
                        out=at[n1:ts], in_=a_sb[0 : ts - n1, i + 1, :]
                    )
                ot = stage.tile([128, C], f32, tag="xa")
                nc.vector.tensor_add(out=ot[:ts], in0=at[:ts], in1=ps[:ts])
                nc.gpsimd.dma_start(out=out_d[t0 : t0 + ts, :], in_=ot[:ts])

    return nc


def _prep_host(inputs):
    import ml_dtypes

    bf16 = ml_dtypes.bfloat16
    f32 = np.float32

    g = {k: np.asarray(v) for k, v in inputs.items()}
    x = g["x"].astype(f32)
    ln1_w, ln1_b = g["ln1_w"].astype(f32), g["ln1_b"].astype(f32)
    ln2_w, ln2_b = g["ln2_w"].astype(f32), g["ln2_b"].astype(f32)
    qkv_w, qkv_b = g["qkv_w"].astype(f32), g["qkv_b"].astype(f32)
    out_w, out_b = g["out_w"].astype(f32), g["out_b"].astype(f32)
    fc1_w, fc1_b = g["fc1_w"].astype(f32), g["fc1_b"].astype(f32)
    fc2_w, fc2_b = g["fc2_w"].astype(f32), g["fc2_b"].astype(f32)
    dw_w, dw_b = g["dw_w"].astype(f32), g["dw_b"].astype(f32)
    temp = float(np.asarray(g["temperature"]))

    # fold LN affine into the following matmul; fold 1/temperature into W_q
    qkv_w2 = qkv_w * ln1_w[None, :]
    qkv_b2 = qkv_b + qkv_w @ ln1_b
    qkv_w2[:C] /= temp
    qkv_b2[:C] /= temp
    fc1_w2 = fc1_w * ln2_w[None, :]
    fc1_b2 = fc1_b + fc1_w @ ln2_b

    shared = {
        "qkvT": np.ascontiguousarray(qkv_w2.T).astype(bf16),
        "qkvb": qkv_b2[None, :].astype(bf16),
        "outT": np.ascontiguousarray(out_w.T).astype(bf16),
        "outb": out_b[None, :].astype(bf16),
        "fc1T": np.ascontiguousarray(fc1_w2.T).astype(bf16),
        "fc1b": fc1_b2[None, :].astype(bf16),
        "fc2T": np.ascontiguousarray(fc2_w.T).astype(bf16),
        "fc2b": fc2_b[None, :].astype(bf16),
        "dww": np.ascontiguousarray(dw_w.reshape(HID, 9)).astype(f32),
        "dwb": dw_b.astype(f32),
        "ident": np.eye(128, dtype=f32).astype(bf16),
    }

    ximg = x.reshape(B, HI, WI, C)
    in_maps = []
    for c in range(NCORES):
        b, qi = c // 4, c % 4
        r0 = RPC * qi
        xe = np.zeros((EXTR, WI, C), f32)
        mask = np.zeros((EXTR, WI), f32)
        for e in range(EXTR):
            r = r0 - 1 + e
            if 0 <= r < HI:
                xe[e] = ximg[b, r]
                mask[e] = 1.0
        m = dict(shared)
        m["xb"] = np.ascontiguousarray(x[b])
        m["xe"] = np.ascontiguousarray(xe.reshape(EXT, C))
        m["mask"] = mask.reshape(EXT).astype(bf16)
        in_maps.append(m)
    return in_maps


def _run(inputs, trace=False):
    from concourse.bass_utils import run_bass_kernel_spmd

    if "nc" not in _CACHE:
        nc = _build_nc()
        nc.finalize()
        _CACHE["nc"] = nc
    nc = _CACHE["nc"]
    in_maps = _prep_host(inputs)
    res = run_bass_kernel_spmd(nc, in_maps, core_ids=list(range(NCORES)), trace=trace)

    x = np.asarray(inputs["x"])
    out = np.zeros((B, NB, C), np.float32)
    for c in range(NCORES):
        b, qi = c // 4, c % 4
        r0 = RPC * qi
        out[b, r0 * WI : (r0 + RPC) * WI, :] = res.results[c]["out"]
    return out.astype(x.dtype, copy=False), res


def kernel(**inputs) -> np.ndarray:
    out, _ = _run(inputs, trace=False)
    return out


# revision 19
# speedup vs baseline: 1.3015x; 1.2503x over previous
"""Trainium2 Bass kernel for a ConvViT-style dense transformer block.

Reference computation (B=2, N=3136=56x56, C=512, 8 heads, hidden 2048):
    x = x + Attn(LN1(x));  x = x + MLP(LN2(x))
    MLP = fc2(gelu(dwconv3x3(fc1(.)) + dw_b))

Sharding: tokens are sharded 8 ways as (batch, 14-image-row) stripes.
Each core computes attention/MLP for its own 14 rows (plus 1 halo row on
each side for the depthwise conv), recomputing K/V projections for its
full batch locally (no collectives).  Host does the (free) scatter/gather.

Key engine assignment:
  - scores/PV/projections/fc1/fc2 and the 3x3 depthwise conv (as 9
    accumulating diagonal matmuls) run on the PE.
  - softmax exp runs on ACT as fused multi-bank [128, ~1536] activations.
  - LN applies run on ACT (Identity with per-partition scale/bias).
  - softmax 1/sum is folded into the PSUM->SBUF copy (DVE reciprocal +
    DMA partition-broadcast).
"""

import numpy as np

# ---------------- problem constants (hardcoded per spec) ----------------
B = 2
HI = 56          # image rows
WI = 56          # image cols
NB = HI * WI     # tokens per batch = 3136
C = 512
NH = 8
HD = 64
F3 = 3 * C       # 1536
HID = 4 * C      # 2048
EPS = 1e-5
NCORES = 8
RPC = HI // 4    # image rows per core = 14
EXTR = RPC + 2   # rows incl halo = 16
EXT = EXTR * WI  # 896 ext tokens
OWN = RPC * WI   # 784 own tokens
QCHS = [(0, 512), (512, 384)]       # attention q-chunks (bank-aligned)
FCHS = [(0, 504), (504, 392)]       # fc1 chunks: rows 0-8 / 9-15

_CACHE = {}


def _btiles():
    # 128-token tiles over the full batch (24 x 128 + 1 x 64)
    return [(i * 128, min(128, NB - i * 128)) for i in range((NB + 127) // 128)]


def _bchunks():
    # 512-token chunks over the full batch (6 x 512 + 1 x 64)
    return [(i * 512, min(512, NB - i * 512)) for i in range((NB + 511) // 512)]


def _build_nc():
    import concourse.bass as bass
    import concourse.bacc as bacc
    import concourse.tile as tile
    from concourse import mybir

    f32 = mybir.dt.float32
    b16 = mybir.dt.bfloat16
    AF = mybir.ActivationFunctionType
    OP = mybir.AluOpType

    nc = bacc.Bacc(trn_type="TRN2")

    # ---- external I/O ----
    xb_d = nc.dram_tensor("xb", [NB, C], f32, kind="ExternalInput")
    xe_d = nc.dram_tensor("xe", [EXT, C], f32, kind="ExternalInput")
    mask_d = nc.dram_tensor("mask", [EXT], b16, kind="ExternalInput")
    qkvT_d = nc.dram_tensor("qkvT", [C, F3], b16, kind="ExternalInput")
    qkvb_d = nc.dram_tensor("qkvb", [1, F3], b16, kind="ExternalInput")
    outT_d = nc.dram_tensor("outT", [C, C], b16, kind="ExternalInput")
    outb_d = nc.dram_tensor("outb", [1, C], b16, kind="ExternalInput")
    fc1T_d = nc.dram_tensor("fc1T", [C, HID], b16, kind="ExternalInput")
    fc1b_d = nc.dram_tensor("fc1b", [1, HID], b16, kind="ExternalInput")
    fc2T_d = nc.dram_tensor("fc2T", [HID, C], b16, kind="ExternalInput")
    fc2b_d = nc.dram_tensor("fc2b", [1, C], b16, kind="ExternalInput")
    dww_d = nc.dram_tensor("dww", [HID, 9], f32, kind="ExternalInput")
    dwb_d = nc.dram_tensor("dwb", [HID], f32, kind="ExternalInput")
    ident_d = nc.dram_tensor("ident", [128, 128], b16, kind="ExternalInput")
    out_d = nc.dram_tensor("out", [OWN, C], f32, kind="ExternalOutput")

    btiles = _btiles()
    bchunks = _bchunks()
    etiles = [(i * 128, 128) for i in range(EXT // 128)]          # 7 x 128
    otiles = [(i * 128, min(128, OWN - i * 128)) for i in range((OWN + 127) // 128)]

    with tile.TileContext(nc) as tc:
        from contextlib import ExitStack

        with ExitStack() as ctx:
            wp = ctx.enter_context(tc.tile_pool(name="wp", bufs=1))
            big = ctx.enter_context(tc.tile_pool(name="big", bufs=1))
            stage = ctx.enter_context(tc.tile_pool(name="stage", bufs=3))
            small = ctx.enter_context(tc.tile_pool(name="small", bufs=4))
            exr = ctx.enter_context(tc.tile_pool(name="exr", bufs=2))
            padp = ctx.enter_context(tc.tile_pool(name="padp", bufs=2))
            dgp = ctx.enter_context(tc.tile_pool(name="dgp", bufs=2))
            # PSUM: score-group ring 2x3 banks + o accumulators 2x1 bank
            psg = ctx.enter_context(tc.tile_pool(name="psg", bufs=2, space="PSUM"))
            pso = ctx.enter_context(tc.tile_pool(name="pso", bufs=1, space="PSUM"))
            _ps_ctr = [0]

            def mk_ps():
                # general-purpose [128,512] psum (projections, out-proj, fc1,
                # fc2, conv) — rotates over the score-group ring's banks,
                # which are free outside the attention inner loop.
                g = psg.tile([128, 1536], f32, tag="sg", name="ps_g")
                return g[:, 0:512]

            # ---------------- constants / weights into SBUF ----------------
            qkvT = wp.tile([128, 4, F3], b16, tag="qkvT")
            nc.sync.dma_start(out=qkvT, in_=qkvT_d[:, :].rearrange("(g p) f -> p g f", p=128))
            qkvb = wp.tile([1, F3], b16, tag="qkvb")
            nc.sync.dma_start(out=qkvb, in_=qkvb_d[:, :])
            outTs = wp.tile([64, 8, C], b16, tag="outTs")
            nc.sync.dma_start(out=outTs, in_=outT_d[:, :].rearrange("(h p) f -> p h f", p=64))
            outb = wp.tile([1, C], b16, tag="outb")
            nc.sync.dma_start(out=outb, in_=outb_d[:, :])
            fc1T = wp.tile([128, 4, HID], b16, tag="fc1T")
            nc.sync.dma_start(out=fc1T, in_=fc1T_d[:, :].rearrange("(g p) f -> p g f", p=128))
            fc1b = wp.tile([1, HID], b16, tag="fc1b")
            nc.sync.dma_start(out=fc1b, in_=fc1b_d[:, :])
            fc2b = wp.tile([1, C], b16, tag="fc2b")
            nc.sync.dma_start(out=fc2b, in_=fc2b_d[:, :])
            dww = wp.tile([128, 16, 9], f32, tag="dww")
            nc.sync.dma_start(out=dww, in_=dww_d[:, :].rearrange("(g p) t -> p g t", p=128))
            dwb = wp.tile([128, 16], f32, tag="dwb")
            nc.sync.dma_start(out=dwb, in_=dwb_d[:].rearrange("(g p) -> p g", p=128))
            maskb = wp.tile([128, EXT], b16, tag="maskb")
            nc.sync.dma_start(
                out=maskb,
                in_=bass.AP(tensor=mask_d[:].tensor, offset=0, ap=[[0, 128], [1, EXT]]),
            )
            ones = wp.tile([1, C], b16, tag="ones")
            nc.vector.memset(ones, 1.0)
            epsc = wp.tile([128, 1], f32, tag="epsc")
            nc.vector.memset(epsc, EPS)
            ident = wp.tile([128, 128], b16, tag="ident")
            nc.sync.dma_start(out=ident, in_=ident_d[:, :])

            # ---------------- LN helpers ----------------
            def layer_norm_tile(xt, ts, lt):
                # stats on DVE; rstd via ACT sqrt + DVE reciprocal;
                # apply on ACT (Identity with per-partition scale/bias)
                st = small.tile([128, 6], f32, tag="st")
                nc.vector.bn_stats(out=st[:ts], in_=xt[:ts])
                mv = small.tile([128, 4], f32, tag="mv")
                nc.vector.bn_aggr(out=mv[:ts, 0:2], in_=st[:ts])
                nc.scalar.activation(
                    out=mv[:ts, 1:2], in_=mv[:ts, 1:2], func=AF.Sqrt,
                    bias=epsc[:ts], scale=1.0,
                )
                nc.vector.reciprocal(out=mv[:ts, 1:2], in_=mv[:ts, 1:2])
                # mv[:,2] = -mu * rstd
                nc.vector.scalar_tensor_tensor(
                    out=mv[:ts, 2:3], in0=mv[:ts, 0:1], scalar=-1.0,
                    in1=mv[:ts, 1:2], op0=OP.mult, op1=OP.mult,
                )
                nc.scalar.activation(
                    out=lt[:ts], in_=xt[:ts], func=AF.Identity,
                    bias=mv[:ts, 2:3], scale=mv[:ts, 1:2],
                )

            def pe_transpose(lt, ts, t0, put):
                # PE-transpose [ts,128] blocks of lt into c-major storage
                for cc in range(4):
                    tp = psg.tile([128, 128], b16, tag="sg", name="tp")
                    nc.tensor.transpose(
                        tp[:, :ts], lt[:ts, cc * 128 : (cc + 1) * 128], ident[:ts, :ts]
                    )
                    put(cc, t0, ts, tp)

            # ---------------- LN1 + K/V projections, interleaved ----------------
            ln1xT = [big.tile([128, NB], b16, tag=f"lx{c}", name=f"ln1xT{c}") for c in range(4)]
            ln1eT = big.tile([128, 4, EXT], b16, tag="le")
            KT = [big.tile([128, NB], b16, tag=f"kt{c}", name=f"KT{c}") for c in range(4)]
            V5 = big.tile([128, len(btiles), 8, 65], b16, tag="v5")
            nc.vector.memset(V5[:, :, :, 64:65], 1.0)

            def put_ln1x(cc, t0, ts, tp):
                nc.vector.tensor_copy(out=ln1xT[cc][:, t0 : t0 + ts], in_=tp[:, :ts])

            def put_ln1e(cc, t0, ts, tp):
                nc.vector.tensor_copy(out=ln1eT[:, cc, t0 : t0 + ts], in_=tp[:, :ts])

            def ln1_tile(src_d, t0, ts, put):
                xt = stage.tile([128, C], f32, tag="xf")
                nc.gpsimd.dma_start(out=xt[:ts], in_=src_d[t0 : t0 + ts, :])
                lt = stage.tile([128, C], b16, tag="xl")
                layer_norm_tile(xt, ts, lt)
                pe_transpose(lt, ts, t0, put)

            def k_proj_chunk(t0, tn):
                for f in range(4):
                    ps = mk_ps()
                    for c in range(4):
                        nc.tensor.matmul(
                            ps[:, :tn],
                            qkvT[:, c, C + f * 128 : C + (f + 1) * 128],
                            ln1xT[c][:, t0 : t0 + tn],
                            start=(c == 0), stop=False,
                        )
                    nc.tensor.matmul(
                        ps[:, :tn],
                        qkvb[:, C + f * 128 : C + (f + 1) * 128],
                        ones[:, :tn],
                        start=False, stop=True,
                    )
                    if f % 2 == 0:
                        nc.vector.tensor_copy(out=KT[f][:, t0 : t0 + tn], in_=ps[:, :tn])
                    else:
                        nc.scalar.activation(
                            out=KT[f][:, t0 : t0 + tn], in_=ps[:, :tn], func=AF.Copy
                        )

            def v_proj_tile(i, t0, ts):
                ps = mk_ps()
                for c in range(4):
                    nc.tensor.matmul(
                        ps[:ts],
                        ln1xT[c][:, t0 : t0 + ts],
                        qkvT[:, c, 2 * C : 3 * C],
                        start=(c == 0), stop=False,
                    )
                nc.tensor.matmul(
                    ps[:ts], ones[:, :ts], qkvb[:, 2 * C : 3 * C],
                    start=False, stop=True,
                )
                if i % 2 == 0:
                    nc.vector.tensor_copy(
                        out=V5[:ts, i, :, 0:64],
                        in_=ps[:ts].rearrange("p (h d) -> p h d", d=64),
                    )
                else:
                    nc.scalar.activation(
                        out=V5[:ts, i, :, 0:64],
                        in_=ps[:ts].rearrange("p (h d) -> p h d", d=64),
                        func=AF.Copy,
                    )

            def k_proj_f(f, t0, tn):
                ps = mk_ps()
                for c in range(4):
                    nc.tensor.matmul(
                        ps[:, :tn],
                        qkvT[:, c, C + f * 128 : C + (f + 1) * 128],
                        ln1xT[c][:, t0 : t0 + tn],
                        start=(c == 0), stop=False,
                    )
                nc.tensor.matmul(
                    ps[:, :tn],
                    qkvb[:, C + f * 128 : C + (f + 1) * 128],
                    ones[:, :tn],
                    start=False, stop=True,
                )
                if f % 2 == 0:
                    nc.vector.tensor_copy(out=KT[f][:, t0 : t0 + tn], in_=ps[:, :tn])
                else:
                    nc.scalar.activation(
                        out=KT[f][:, t0 : t0 + tn], in_=ps[:, :tn], func=AF.Copy
                    )

            # fine-grained interleave: each LN1 tile is followed by ~2
            # projection units of the previous chunk, keeping the PE dense.
            proj_units = []
            for i, (t0, ts) in enumerate(btiles):
                ci = t0 // 512
                c0, cn = bchunks[ci]
                if t0 + ts == c0 + cn:  # chunk complete after this tile
                    for f in range(4):
                        proj_units.append(("k", f, c0, cn))
                    for k in range(ci * 4, min(ci * 4 + 4, len(btiles))):
                        proj_units.append(("v", k))
            pu = [0]

            def drain_proj(n):
                while n > 0 and pu[0] < len(proj_units):
                    u = proj_units[pu[0]]
                    pu[0] += 1
                    if u[0] == "k":
                        k_proj_f(u[1], u[2], u[3])
                    else:
                        v_proj_tile(u[1], *btiles[u[1]])
                    n -= 1

            done_tiles = 0
            for i, (t0, ts) in enumerate(btiles):
                ln1_tile(xb_d, t0, ts, put_ln1x)
                done_tiles += 1
                if done_tiles > 4:
                    drain_proj(2)

            # LN1 on ext tokens + Q projection, similarly interleaved
            QT = big.tile([128, 4, EXT], b16, tag="qt")

            def q_proj_f(f, q0, qn):
                ps = mk_ps()
                for c in range(4):
                    nc.tensor.matmul(
                        ps[:, :qn],
                        qkvT[:, c, f * 128 : (f + 1) * 128],
                        ln1eT[:, c, q0 : q0 + qn],
                        start=(c == 0), stop=False,
                    )
                nc.tensor.matmul(
                    ps[:, :qn], qkvb[:, f * 128 : (f + 1) * 128],
                    ones[:, :qn], start=False, stop=True,
                )
                nc.vector.tensor_copy(out=QT[:, f, q0 : q0 + qn], in_=ps[:, :qn])

            for t0, ts in etiles[0:4]:
                ln1_tile(xe_d, t0, ts, put_ln1e)
                drain_proj(2)
            for i, (t0, ts) in enumerate(etiles[4:7]):
                ln1_tile(xe_d, t0, ts, put_ln1e)
                drain_proj(2)
                q_proj_f(i, *QCHS[0])
            drain_proj(100)
            q_proj_f(3, *QCHS[0])
            for f in range(4):
                q_proj_f(f, *QCHS[1])

            # ---------------- attention ----------------
            # Per (qc, pr): stream of 50 score-matmul outputs (kt-major,
            # head A then B) packed into 3-bank psum group tiles; one fused
            # exp per group; PV matmuls consume the bf16 exp output.
            oTs = big.tile([64, 8, EXT], b16, tag="oTs")
            a_sb = big.tile([128, 7, C], b16, tag="a_sb")
            # reuses ln1eT's slot — dead after the Q projection
            ln2aT = big.tile([128, 4, EXT], b16, tag="le")

            def put_ln2a(cc, t0, ts, tp):
                nc.vector.tensor_copy(out=ln2aT[:, cc, t0 : t0 + ts], in_=tp[:, :ts])

            def outproj_etile(i, t0, ts):
                # out-proj + residual only; LN2 happens post-attention so no
                # ACT table switch lands inside the exp window.
                ps = mk_ps()
                for h in range(8):
                    nc.tensor.matmul(
                        ps, oTs[:, h, t0 : t0 + ts], outTs[:, h, :],
                        start=(h == 0), stop=False,
                    )
                nc.tensor.matmul(ps, ones[:, :ts], outb, start=False, stop=True)
                xt = stage.tile([128, C], f32, tag="xf")
                nc.gpsimd.dma_start(out=xt[:ts], in_=xe_d[t0 : t0 + ts, :])
                nc.vector.tensor_add(out=a_sb[:ts, i, :], in0=xt[:ts], in1=ps[:ts])

            def ln2_etile(i, t0, ts):
                lt = stage.tile([128, C], b16, tag="xl")
                layer_norm_tile(a_sb[:, i, :], ts, lt)
                pe_transpose(lt, ts, t0, put_ln2a)

            # softmax-sum staging for the deferred normalize
            srows = big.tile([1, 8, 512], b16, tag="srows")
            rrows = big.tile([1, 8, 512], b16, tag="rrows")
            pending = []  # (slot, head, q0, qn) awaiting recip+broadcast+norm

            def emit_recips(todo):
                for slot, h, q0, qn in todo:
                    with nc.allow_low_precision(reason="1/softmax-sum bf16"):
                        nc.vector.reciprocal(
                            out=rrows[0:1, slot, 0:qn], in_=srows[0:1, slot, 0:qn]
                        )

            def emit_norms(todo):
                for slot, h, q0, qn in todo:
                    rb = mk_ps()
                    nc.tensor.matmul(
                        rb[0:64, 0:qn], ones[:, 0:64], rrows[0:1, slot, 0:qn],
                        start=True, stop=True,
                    )
                    dst = oTs[:, h, q0 : q0 + qn]
                    nc.vector.scalar_tensor_tensor(
                        out=dst, in0=dst, scalar=1.0, in1=rb[0:64, 0:qn],
                        op0=OP.bypass, op1=OP.mult,
                    )

            nkt = len(btiles)
            # qc1 runs first so its out-proj etiles (4-6) can fill qc0's
            # pr-boundary gaps; normalize is deferred off the PE path.
            for qci, (q0, qn) in enumerate((QCHS[1], QCHS[0])):
                for pr in range(4):
                    todo = pending[:]
                    del pending[:]
                    oA = pso.tile([128, 512], f32, tag="oA")
                    oB = pso.tile([128, 512], f32, tag="oB")
                    # out j = 2*kt + (0 if head A else 1); groups of 3
                    njs = 2 * nkt
                    ngrp = (njs + 2) // 3
                    grp_tiles = {}
                    ex_tiles = {}
                    for g in range(ngrp):
                        js = list(range(3 * g, min(3 * g + 3, njs)))
                        # score matmuls for this group's outputs
                        for j in js:
                            kt, hb = j // 2, j % 2
                            k0, kn = btiles[kt]
                            if j % 3 == 0:
                                grp_tiles[g] = psg.tile(
                                    [128, 1536], f32, tag="sg", name="sg",
                                )
                            gt = grp_tiles[j // 3]
                            col = (j % 3) * 512
                            nc.tensor.matmul(
                                gt[0:kn, col : col + qn],
                                KT[pr][64 * hb : 64 * hb + 64, k0 : k0 + kn],
                                QT[64 * hb : 64 * hb + 64, pr, q0 : q0 + qn],
                                start=True, stop=True,
                                tile_position=(64 * hb, 0),
                            )
                        # fused exp over the whole group (3D AP skips pad)
                        gt = grp_tiles[g]
                        nj = len(js)
                        ex = exr.tile([128, 1536], b16, tag="ex", name="ex")
                        ex_tiles[g] = ex
                        gv = gt.rearrange("p (s c) -> p s c", c=512)[:, 0:nj, 0:qn]
                        xv = ex.rearrange("p (s c) -> p s c", c=512)[:, 0:nj, 0:qn]
                        nc.scalar.activation(out=xv, in_=gv, func=AF.Exp)
                        # PV matmuls for this group's outputs
                        for j in js:
                            kt, hb = j // 2, j % 2
                            k0, kn = btiles[kt]
                            col = (j % 3) * 512
                            o = oB if hb else oA
                            nc.tensor.matmul(
                                o[0:65, 0:qn],
                                V5[:kn, kt, 2 * pr + hb, :],
                                ex_tiles[j // 3][0:kn, col : col + qn],
                                start=(kt == 0), stop=(kt == nkt - 1),
                            )
                        # deferred normalize of the previous pr, emitted at
                        # slack points so the PE never waits on it
                        if g == 1:
                            emit_recips(todo)
                        elif g == 3:
                            emit_norms(todo)
                    # at pr end: evacuate o and the exp-sums with two fast
                    # DVE copies; everything slow is deferred.
                    for hb, o in ((0, oA), (1, oB)):
                        h = 2 * pr + hb
                        slot = pr * 2 + hb
                        nc.vector.tensor_copy(
                            out=oTs[:, h, q0 : q0 + qn], in_=o[0:64, 0:qn]
                        )
                        nc.vector.tensor_copy(
                            out=srows[0:1, slot, 0:qn], in_=o[64:65, 0:qn]
                        )
                        pending.append((slot, h, q0, qn))
                    # out-proj of qc1's etiles fills qc0's pr boundaries
                    if qci == 1 and pr < 3:
                        outproj_etile(4 + pr, *etiles[4 + pr])

            emit_recips(pending)
            emit_norms(pending)
            del pending[:]
            for i in range(4):
                outproj_etile(i, *etiles[i])
            for i in range(7):
                ln2_etile(i, *etiles[i])

            # ---------------- MLP: fc1 -> scatter -> PE dwconv -> gelu ----------------
            # fc2 weights arrive late, into slots KT vacated after attention
            fc2Ta = big.tile([128, 8, C], b16, tag="kt0")
            nc.gpsimd.dma_start(
                out=fc2Ta, in_=fc2T_d[0:1024, :].rearrange("(g p) f -> p g f", p=128)
            )
            fc2Tb = big.tile([128, 8, C], b16, tag="kt1")
            nc.gpsimd.dma_start(
                out=fc2Tb, in_=fc2T_d[1024:2048, :].rearrange("(g p) f -> p g f", p=128)
            )
            ghT = [big.tile([128, 4, OWN], b16, tag=f"lx{k}", name=f"ghT{k}") for k in range(4)]
            SPAN = RPC * (WI + 2)          # 812 flat conv span (2 junk cols/row)
            HSPAN = SPAN // 2              # 406 = 7 rows
            PADW = EXTR * (WI + 2) + 2     # 930: +2 guard for last-tap reads
            for g in range(16):
                pad = padp.tile([128, PADW], b16, tag="pad")
                padv = pad[:, : PADW - 2].rearrange("p (r x) -> p r x", x=WI + 2)
                nc.vector.memset(pad[:, PADW - 2 :], 0.0)
                nc.vector.memset(padv[:, :, 0:1], 0.0)
                nc.vector.memset(padv[:, :, WI + 1 : WI + 2], 0.0)
                for fi, (f0, fn) in enumerate(FCHS):
                    ps = mk_ps()
                    for c in range(4):
                        nc.tensor.matmul(
                            ps[:, :fn],
                            fc1T[:, c, g * 128 : (g + 1) * 128],
                            ln2aT[:, c, f0 : f0 + fn],
                            start=(c == 0), stop=False,
                        )
                    nc.tensor.matmul(
                        ps[:, :fn], fc1b[:, g * 128 : (g + 1) * 128],
                        ones[:, :fn], start=False, stop=True,
                    )
                    r0 = f0 // WI
                    nr = fn // WI
                    nc.vector.scalar_tensor_tensor(
                        out=padv[:, r0 : r0 + nr, 1 : WI + 1],
                        in0=ps[:, :fn].rearrange("p (r x) -> p r x", x=WI),
                        scalar=1.0,
                        in1=maskb[:, f0 : f0 + fn].rearrange("p (r x) -> p r x", x=WI),
                        op0=OP.bypass, op1=OP.mult,
                    )
                # diagonal weight matrices for this group's 9 taps
                dg = dgp.tile([128, 9, 128], b16, tag="dg")
                for tap in range(9):
                    nc.vector.tensor_scalar_mul(
                        out=dg[:, tap, :], in0=ident, scalar1=dww[:, g, tap : tap + 1]
                    )
                # 3x3 depthwise conv: 9 accumulating diag matmuls per chunk
                for ch in range(2):
                    cps = psg.tile([128, 1536], f32, tag="sg", name="cps")
                    for dy in range(3):
                        for dx in range(3):
                            tap = 3 * dy + dx
                            off = dy * (WI + 2) + dx + ch * HSPAN
                            nc.tensor.matmul(
                                cps[:, 0:HSPAN],
                                dg[:, tap, :],
                                pad[:, off : off + HSPAN],
                                start=(tap == 0), stop=(tap == 8),
                            )
                    # gelu(conv + dwb) straight out of PSUM, skipping the
                    # 2 junk cols per row
                    cv = cps[:, 0:HSPAN].rearrange("p (r x) -> p r x", x=WI + 2)
                    nc.scalar.activation(
                        out=ghT[g // 4][:, g % 4, ch * 392 : (ch + 1) * 392],
                        in_=cv[:, :, 0:WI],
                        func=AF.Gelu, bias=dwb[:, g : g + 1], scale=1.0,
                    )

            # ---------------- fc2 + final residual ----------------
            for i, (t0, ts) in enumerate(otiles):
                ps = mk_ps()
                for k in range(16):
                    f2 = fc2Ta[:, k, :] if k < 8 else fc2Tb[:, k - 8, :]
                    nc.tensor.matmul(
                        ps[:ts],
                        ghT[k // 4][:, k % 4, t0 : t0 + ts],
                        f2,
                        start=(k == 0), stop=False,
                    )
                nc.tensor.matmul(ps[:ts], ones[:, :ts], fc2b, start=False, stop=True)
                at = stage.tile([128, C], f32, tag="xf")
                n1 = min(ts, 128 - WI)  # rows from a tile i (partitions WI..)
                nc.gpsimd.dma_start(out=at[:n1], in_=a_sb[WI : WI + n1, i, :])
                if ts > n1:
                    nc.gpsimd.dma_start(
                        out=at[n1:ts], in_=a_sb[0 : ts - n1, i + 1, :]
                    )
                ot = stage.tile([128, C], f32, tag="xa")
                nc.vector.tensor_add(out=ot[:ts], in0=at[:ts], in1=ps[:ts])
                nc.gpsimd.dma_start(out=out_d[t0 : t0 + ts, :], in_=ot[:ts])

    return nc


def _prep_host(inputs):
    import ml_dtypes

    bf16 = ml_dtypes.bfloat16
    f32 = np.float32

    g = {k: np.asarray(v) for k, v in inputs.items()}
    x = g["x"].astype(f32)
    ln1_w, ln1_b = g["ln1_w"].astype(f32), g["ln1_b"].astype(f32)
    ln2_w, ln2_b = g["ln2_w"].astype(f32), g["ln2_b"].astype(f32)
    qkv_w, qkv_b = g["qkv_w"].astype(f32), g["qkv_b"].astype(f32)
    out_w, out_b = g["out_w"].astype(f32), g["out_b"].astype(f32)
    fc1_w, fc1_b = g["fc1_w"].astype(f32), g["fc1_b"].astype(f32)
    fc2_w, fc2_b = g["fc2_w"].astype(f32), g["fc2_b"].astype(f32)
    dw_w, dw_b = g["dw_w"].astype(f32), g["dw_b"].astype(f32)
    temp = float(np.asarray(g["temperature"]))

    # fold LN affine into the following matmul; fold 1/temperature into W_q
    qkv_w2 = qkv_w * ln1_w[None, :]
    qkv_b2 = qkv_b + qkv_w @ ln1_b
    qkv_w2[:C] /= temp
    qkv_b2[:C] /= temp
    fc1_w2 = fc1_w * ln2_w[None, :]
    fc1_b2 = fc1_b + fc1_w @ ln2_b

    shared = {
        "qkvT": np.ascontiguousarray(qkv_w2.T).astype(bf16),
        "qkvb": qkv_b2[None, :].astype(bf16),
        "outT": np.ascontiguousarray(out_w.T).astype(bf16),
        "outb": out_b[None, :].astype(bf16),
        "fc1T": np.ascontiguousarray(fc1_w2.T).astype(bf16),
        "fc1b": fc1_b2[None, :].astype(bf16),
        "fc2T": np.ascontiguousarray(fc2_w.T).astype(bf16),
        "fc2b": fc2_b[None, :].astype(bf16),
        "dww": np.ascontiguousarray(dw_w.reshape(HID, 9)).astype(f32),
        "dwb": dw_b.astype(f32),
        "ident": np.eye(128, dtype=f32).astype(bf16),
    }

    ximg = x.reshape(B, HI, WI, C)
    in_maps = []
    for c in range(NCORES):
        b, qi = c // 4, c % 4
        r0 = RPC * qi
        xe = np.zeros((EXTR, WI, C), f32)
        mask = np.zeros((EXTR, WI), f32)
        for e in range(EXTR):
            r = r0 - 1 + e
            if 0 <= r < HI:
                xe[e] = ximg[b, r]
                mask[e] = 1.0
        m = dict(shared)
        m["xb"] = np.ascontiguousarray(x[b])
        m["xe"] = np.ascontiguousarray(xe.reshape(EXT, C))
        m["mask"] = mask.reshape(EXT).astype(bf16)
        in_maps.append(m)
    return in_maps


def _run(inputs, trace=False):
    from concourse.bass_utils import run_bass_kernel_spmd

    if "nc" not in _CACHE:
        nc = _build_nc()
        nc.finalize()
        _CACHE["nc"] = nc
    nc = _CACHE["nc"]
    in_maps = _prep_host(inputs)
    res = run_bass_kernel_spmd(nc, in_maps, core_ids=list(range(NCORES)), trace=trace)

    x = np.asarray(inputs["x"])
    out = np.zeros((B, NB, C), np.float32)
    for c in range(NCORES):
        b, qi = c // 4, c % 4
        r0 = RPC * qi
        out[b, r0 * WI : (r0 + RPC) * WI, :] = res.results[c]["out"]
    return out.astype(x.dtype, copy=False), res


def kernel(**inputs) -> np.ndarray:
    out, _ = _run(inputs, trace=False)
    return out
